# revision 17
# baseline (speedup 1.0000x reference)
"""Trainium2 Bass kernel for nn_ContextEncoder.

Pipeline (per sample b): feature transform tanh(X @ W_t.T + b_t), a
"bidirectional" LSTM where both directions run forward (matching the
reference), attention pooling against the last hidden state, and a
context norm over the flattened (d, 2h) vector.

Sharding: data-parallel over b (16 samples -> 2 per core on 8 cores).
Each core runs 128 independent sequences (2 b x 64 d) of length T=128.

v2 schedule notes:
  - per-step PSUM gate groups (1 bank per dir, bufs=2); xW matmuls for
    step t+1 are emitted during step t so the PE never waits on the
    WAR hazard against the sigmoid reads.
  - PE transposes of h_t (for the attention tail) are emitted during
    step t+1, after the recurrence matmuls, keeping them off the
    critical h -> matmul path. They accumulate 4 steps per PSUM bank
    and are evacuated to the [j, t, 2h] slab by the GpSimd engine.
  - sigma_f * c_prev runs on GpSimd (off the serial path); the tg/u/cn
    chain stays on DVE.
  - tail: logits via DVE/GpSimd-split multiply + pair tree + packed
    tensor_reduce (contiguous fp32 logits), softmax on ACT, weighted
    pooling split across DVE/ACT/GpSimd, final reduction via
    tensor_reduce over a transposed view.
"""

import sys

for _p in ("/opt/trn_rl_repo", "/root/.axon_site/_ro/trn_rl_repo"):
    if _p not in sys.path:
        sys.path.append(_p)

import numpy as np

import concourse.bass as bass
import concourse.bacc as bacc
import concourse.tile as tile
from concourse import mybir
from concourse.bass_utils import run_bass_kernel_spmd

BF16NP = np.float16
F32 = mybir.dt.float32
BF16 = mybir.dt.float16
AF = mybir.ActivationFunctionType
ALU = mybir.AluOpType

B, T, D, NF = 16, 128, 64, 32
TS, H = 64, 128
NCORES = 8
BLOC = B // NCORES          # 2 samples per core
J = BLOC * D                # 128 sequences per core
R = J * T                   # 16384 (t, b, d) columns
G4 = 4 * H                  # 512 gates per direction
PERM = (0, 1, 3, 2)         # torch gate order (i,f,g,o) -> (i,f,o,g)
NORM_N = D * 2 * H          # 16384 context-norm elements per sample
TCH = 1024                  # transform chunk columns
# tail work split: DVE ~3.8x faster than GpSimd per element
TD = 104                    # t-range handled by DVE in prod/tree ops


def emit(tc, ins, outs):
    nc = tc.nc
    XT, WTT, BT = ins["XT"], ins["WTT"], ins["BT"]
    WIH, WHH, ONES = ins["WIH"], ins["WHH"], ins["ONES"]
    DW, DB = ins["DW"], ins["DB"]
    OUT = outs["OUT"]

    with (
        tc.tile_pool(name="consts", bufs=1) as consts,
        tc.tile_pool(name="sgpool", bufs=2) as sgpool,
        tc.tile_pool(name="small", bufs=2) as small,
    ):
        # ---- constants / weights ----
        wtt = consts.tile([NF, TS], BF16)
        nc.sync.dma_start(wtt, WTT)
        bt = consts.tile([TS, 1], F32)
        nc.sync.dma_start(bt, BT)
        wih = consts.tile([TS + 1, 2, G4], BF16)
        whh = consts.tile([H, 2, G4], BF16)
        # HT: attention layout [j, t, 2h] filled by per-step PE transposes
        ht = consts.tile([J, T, 2 * H], BF16)
        idn = consts.tile([H, H], BF16)

        with (
            tc.tile_pool(name="xs2p", bufs=1) as xs2p,
            tc.tile_pool(name="hslabs", bufs=1) as hslabs,
            tc.tile_pool(name="xtp", bufs=2) as xtp,
        ):
            # ---- feature transform: xs2[0:64, (t,b,d)] = tanh(Wt @ X.T + bt)
            # Computed in 512-col (4-step) chunks, interleaved into the
            # recurrence: only chunks 0-2 run up front.
            xs2 = xs2p.tile([TS + 1, R], BF16)
            BLK = 4096
            xts = {}

            def emit_xt_dma(blk):
                xt = xtp.tile([NF, BLK], BF16, tag="xt")
                nc.sync.dma_start(xt, XT[:, blk * BLK : (blk + 1) * BLK])
                xts[blk] = xt

            emit_xt_dma(0)
            nc.sync.dma_start(xs2[TS : TS + 1, :], ONES)
            nc.sync.dma_start(wih, WIH)
            nc.sync.dma_start(whh, WHH)
            nc.sync.dma_start(idn, ins["IDN"])
            # h history slabs: transpose sources are never recycled
            hsl = [hslabs.tile([H, R], BF16, tag=f"hs{d}", name=f"hs{d}")
                   for d in range(2)]
            # per-dir [parity, {tg, c}] state tiles: uc2 reads (tg@p, c@1-p)
            # as one paged AP, pairing with sigma chunks (i, f)
            ctile = [hslabs.tile([H, 2, 2, J], BF16, tag=f"ct{d}",
                                 name=f"ct{d}") for d in range(2)]
            h_prev = [None, None]
            for d in range(2):
                h0 = hslabs.tile([H, J], BF16, tag=f"hz{d}", name=f"hz{d}")
                nc.vector.memset(h0, 0.0)
                nc.vector.memset(ctile[d][:, 1, 1, :], 0.0)
                h_prev[d] = h0

            # ---- recurrence ----
            # Per-step PSUM gate tiles (1 bank/dir, double buffered).
            # trp: 4 steps x 2 dirs of transposed h per bank.
            with (
                tc.tile_pool(name="gates", bufs=2, space="PSUM") as gates,
                tc.tile_pool(name="trp", bufs=3, space="PSUM") as trp,
                tc.tile_pool(name="tfp", bufs=1, space="PSUM") as tfp,
            ):
                def emit_tf_mm(c):
                    pz = tfp.tile([TS, 512], F32, tag="pz")
                    nc.tensor.matmul(
                        pz, lhsT=wtt,
                        rhs=xts[c // 8][:, (c % 8) * 512 : (c % 8 + 1) * 512],
                        start=True, stop=True)
                    return pz

                def emit_tf_act(c, pz):
                    nc.scalar.activation(
                        out=xs2[0:TS, c * 512 : (c + 1) * 512],
                        in_=pz, func=AF.Tanh, bias=bt, scale=1.0)

                def emit_xw(t, pg):
                    rhs_x = xs2[:, t * J : (t + 1) * J]
                    for d in range(2):
                        for c in range(4):
                            nc.tensor.matmul(
                                pg[d][:, c, :],
                                lhsT=wih[:, d, c * H : (c + 1) * H],
                                rhs=rhs_x, start=(c == 0), stop=False,
                            )

                def new_group():
                    return [gates.tile([H, 4, J], F32, tag=f"g{d}",
                                       name=f"pg{d}") for d in range(2)]

                for c in range(3):
                    emit_tf_act(c, emit_tf_mm(c))
                psg = new_group()
                emit_xw(0, psg)
                psg_next = None
                trtile = [None]
                tr_keep = []
                for t in range(T):
                    # prefetch the next XT block / transform chunk state
                    if t in (8, 40, 72):
                        emit_xt_dma(t // 32 + 1)
                    tf_c = (t + 7) // 4 if (t % 4 == 1) else None
                    if tf_c is not None and not (3 <= tf_c < 32):
                        tf_c = None
                    # recurrence matmuls for step t
                    for d in range(2):
                        for c in range(4):
                            nc.tensor.matmul(
                                psg[d][:, c, :],
                                lhsT=whh[:, d, c * H : (c + 1) * H],
                                rhs=h_prev[d], start=False, stop=(c == 3),
                            )
                    # xW for step t+1 fills the PE while ACT/DVE work on t
                    if t < T - 1:
                        psg_next = new_group()
                        emit_xw(t + 1, psg_next)
                    # transposes of h_{t-1} (off critical path, last on PE)
                    if t >= 1:
                        s = t - 1
                        if s % 4 == 0:
                            trtile[0] = trp.tile([J, 8, H], BF16, tag="tr",
                                                 name="ptr")
                            tr_keep.append(trtile[0])
                        for d in range(2):
                            nc.tensor.transpose(
                                trtile[0][:, (s % 4) * 2 + d, :],
                                hsl[d][:, s * J : (s + 1) * J], idn)
                    tf_pz = emit_tf_mm(tf_c) if tf_c is not None else None

                    # elementwise tail of step t
                    p = t % 2
                    sg4, uc2, tc_ = {}, {}, {}
                    for d in range(2):
                        sg4[d] = sgpool.tile([H, 4, J], BF16, tag=f"sg{d}",
                                             name=f"sg{d}")
                        nc.scalar.activation(out=sg4[d], in_=psg[d],
                                             func=AF.Sigmoid)
                    for d in range(2):
                        # tg = 2*sigmoid(2g) - 1 into the parity-p tg slot
                        nc.vector.tensor_scalar(ctile[d][:, p, 0, :],
                                                sg4[d][:, 3, :], 2.0, -1.0,
                                                op0=ALU.mult, op1=ALU.add)
                    for d in range(2):
                        # c2 = sg_f * c_prev (off the serial path)
                        uc2[d] = small.tile([H, 2, J], BF16, tag=f"uc{d}",
                                            name=f"uc{d}")
                        nc.vector.tensor_mul(uc2[d][:, 1, :], sg4[d][:, 1, :],
                                             ctile[d][:, 1 - p, 1, :])
                    for d in range(2):
                        # u = sg_i * tg (serial path)
                        nc.vector.tensor_mul(uc2[d][:, 0, :], sg4[d][:, 0, :],
                                             ctile[d][:, p, 0, :])
                    for d in range(2):
                        nc.vector.tensor_add(ctile[d][:, p, 1, :],
                                             uc2[d][:, 0, :], uc2[d][:, 1, :])
                    for d in range(2):
                        tc_[d] = small.tile([H, J], BF16, tag=f"tc{d}",
                                            name=f"tc{d}")
                        nc.scalar.activation(out=tc_[d], in_=ctile[d][:, p, 1, :],
                                             func=AF.Tanh)
                    # evacuate two steps of transposed h on the (in-order)
                    # scalar engine right after tanh: lands in ACT's idle
                    # window each even step, never preempting the DVE chain
                    if tf_c is not None:
                        emit_tf_act(tf_c, tf_pz)
                    for d in range(2):
                        hn = hsl[d][:, t * J : (t + 1) * J]
                        nc.vector.tensor_mul(hn, sg4[d][:, 2, :], tc_[d])
                        h_prev[d] = hn
                    # evacuate step t-1's transposed h (one [J, 2H] piece per
                    # step, after hn: runs in the DVE idle window)
                    if t >= 1:
                        s = t - 1
                        sl = (s % 4) * 2
                        nc.vector.tensor_copy(
                            ht[:, s, :],
                            trtile[0][:, sl : sl + 2, :].rearrange(
                                "j a b -> j (a b)"),
                        )
                    psg = psg_next
                # final transposes for step T-1 (slot s%4 = 3 of current tile)
                s = T - 1
                for d in range(2):
                    nc.tensor.transpose(trtile[0][:, (s % 4) * 2 + d, :],
                                        hsl[d][:, s * J : (s + 1) * J], idn)
                nc.vector.tensor_copy(
                    ht[:, T - 1, :],
                    trtile[0][:, 6:8, :].rearrange("j a b -> j (a b)"),
                )

        # ---- tail: attention pooling + context norm ----
        with (
            tc.tile_pool(name="tailp", bufs=1) as tailp,
            tc.tile_pool(name="tailps", bufs=1, space="PSUM") as tailps,
        ):
            htj = ht[:, T - 1, :]  # [J, 2H] last hidden state
            def htj_bcast(t0, t1):
                return bass.AP(
                    tensor=htj.tensor, offset=htj.offset,
                    ap=[list(htj.ap[0]), [0, t1 - t0], list(htj.ap[-1])],
                )
            prod = tailp.tile([J, T, 2 * H], BF16)
            nc.vector.tensor_mul(prod, ht, htj_bcast(0, T))
            # pair tree over the 2h dim, bf16, t-split DVE/GpSimd
            pp0 = tailp.tile([J, T, 128], BF16)
            def lvl(dst, a, b):
                nc.vector.tensor_add(dst, a, b)
            lvl(pp0, prod[:, :, 0:128], prod[:, :, 128:256])
            lvl(prod[:, :, 0:64], pp0[:, :, 0:64], pp0[:, :, 64:128])
            lvl(pp0[:, :, 0:32], prod[:, :, 0:32], prod[:, :, 32:64])
            lvl(prod[:, :, 0:16], pp0[:, :, 0:16], pp0[:, :, 16:32])
            # packed fp32 logits via innermost tensor_reduce
            lt = tailp.tile([J, T], F32)
            nc.vector.tensor_reduce(lt, prod[:, :, 0:16],
                                    axis=mybir.AxisListType.X, op=ALU.add)
            mx = tailp.tile([J, 1], F32)
            nc.vector.tensor_reduce(mx, lt, axis=mybir.AxisListType.X, op=ALU.max)
            mxn = tailp.tile([J, 1], F32)
            nc.vector.tensor_scalar_mul(mxn, mx, -1.0)
            ew = tailp.tile([J, T], F32)
            dsum = tailp.tile([J, 1], F32)
            nc.scalar.activation(out=ew, in_=lt, func=AF.Exp, bias=mxn,
                                 scale=1.0, accum_out=dsum)
            rd = tailp.tile([J, 1], F32)
            nc.vector.reciprocal(rd, dsum)
            nc.vector.tensor_scalar_mul(ew, ew, rd)  # softmax weights in place
            # weighted pooling: per-t scale ops split DVE/ACT/GpSimd
            prod2 = tailp.tile([J, T, 2 * H], BF16, tag="prod")  # reuse slab
            act_ts = set(range(43, 64)) | set(range(107, 128))
            for tt in list(range(0, 43)) + list(range(64, 107)) + sorted(act_ts):
                if tt in act_ts:
                    nc.scalar.mul(prod2[:, tt, :], ht[:, tt, :],
                                  ew[:, tt : tt + 1])
                else:
                    nc.vector.tensor_scalar_mul(prod2[:, tt, :], ht[:, tt, :],
                                                ew[:, tt : tt + 1])
            # pair tree over t: bf16 down to 16 steps, then fp32
            def tlvl(dst, a, b, n):
                nc.vector.tensor_add(dst, a, b)
            s64 = pp0.rearrange("j t b -> j (t b)").rearrange(
                "j (t b) -> j t b", b=2 * H)  # [J, 64, 2H] view over pp0
            nc.vector.tensor_add(s64[:, 0:43, :], prod2[:, 0:43, :],
                                 prod2[:, 64:107, :])
            nc.vector.tensor_add(s64[:, 43:64, :], prod2[:, 43:64, :],
                                 prod2[:, 107:128, :])
            tlvl(prod2[:, 0:32, :], s64[:, 0:32, :], s64[:, 32:64, :], 32)
            tlvl(s64[:, 0:16, :], prod2[:, 0:16, :], prod2[:, 16:32, :], 16)
            ptrf = tailp.tile([J, 8, 2 * H], F32)
            tlvl(ptrf, s64[:, 0:8, :], s64[:, 8:16, :], 8)
            nc.vector.tensor_add(ptrf[:, 0:4, :], ptrf[:, 0:4, :],
                                 ptrf[:, 4:8, :])
            nc.vector.tensor_add(ptrf[:, 0:2, :], ptrf[:, 0:2, :],
                                 ptrf[:, 2:4, :])
            pooled = tailp.tile([J, 2 * H], F32)
            nc.vector.tensor_add(pooled, ptrf[:, 0, :], ptrf[:, 1, :])

            # context norm across each sample's (d, 2h) block
            pooled2 = tailp.tile([J, 2 * H], F32)
            nc.scalar.activation(out=pooled2, in_=pooled, func=AF.Square)
            sel = tailp.tile([J, BLOC], F32)
            nc.sync.dma_start(sel, ins["SEL"])
            pstat = tailps.tile([BLOC, 2 * G4], F32, tag="stats")
            nc.tensor.matmul(pstat[:, 0 : 2 * H], lhsT=sel, rhs=pooled,
                             start=True, stop=False)
            nc.tensor.matmul(pstat[:, 2 * H : 4 * H], lhsT=sel, rhs=pooled2,
                             start=False, stop=True)
            s1 = tailp.tile([BLOC, 1], F32)
            nc.vector.tensor_reduce(s1, pstat[:, 0 : 2 * H],
                                    axis=mybir.AxisListType.X, op=ALU.add)
            s2 = tailp.tile([BLOC, 1], F32)
            nc.vector.tensor_reduce(s2, pstat[:, 2 * H : 4 * H],
                                    axis=mybir.AxisListType.X, op=ALU.add)
            stats2 = tailp.tile([BLOC, 2], F32)
            nc.scalar.mul(stats2[:, 0:1], s1, 1.0 / NORM_N)      # mean
            q = tailp.tile([BLOC, 1], F32)
            nc.vector.tensor_mul(q, s1, stats2[:, 0:1])          # sum*mean
            v = tailp.tile([BLOC, 1], F32)
            nc.vector.tensor_tensor(v, s2, q, op=ALU.subtract)
            sd = tailp.tile([BLOC, 1], F32)
            nc.scalar.activation(out=sd, in_=v, func=AF.Sqrt,
                                 scale=1.0 / (NORM_N - 1))
            nc.vector.reciprocal(stats2[:, 1:2], sd)
            selt = tailp.tile([BLOC, J], F32)
            nc.sync.dma_start(selt, ins["SELT"])
            pmb = tailps.tile([J, 2], F32, tag="mb")
            nc.tensor.matmul(pmb, lhsT=selt, rhs=stats2, start=True, stop=True)
            mb = tailp.tile([J, 2], F32)
            nc.vector.tensor_copy(mb, pmb)
            dwt = tailp.tile([J, 2 * H], F32)
            nc.sync.dma_start(dwt[0:D, :], DW)
            nc.sync.dma_start(dwt[D:J, :], DW)
            dbt = tailp.tile([J, 2 * H], F32)
            nc.sync.dma_start(dbt[0:D, :], DB)
            nc.sync.dma_start(dbt[D:J, :], DB)
            t1 = tailp.tile([J, 2 * H], F32)
            nc.vector.tensor_scalar(t1, pooled, mb[:, 0:1], mb[:, 1:2],
                                    op0=ALU.subtract, op1=ALU.mult)
            t2 = tailp.tile([J, 2 * H], F32)
            nc.vector.tensor_mul(t2, t1, dwt)
            t3 = tailp.tile([J, 2 * H], F32)
            nc.vector.tensor_add(t3, t2, dbt)
            nc.sync.dma_start(OUT, t3)


def build_program():
    nc = bacc.Bacc("TRN2", target_bir_lowering=False, debug=False)
    ins = {
        "XT": nc.dram_tensor("XT", [NF, R], BF16, kind="ExternalInput").ap(),
        "WTT": nc.dram_tensor("WTT", [NF, TS], BF16, kind="ExternalInput").ap(),
        "BT": nc.dram_tensor("BT", [TS, 1], F32, kind="ExternalInput").ap(),
        "WIH": nc.dram_tensor("WIH", [TS + 1, 2, G4], BF16, kind="ExternalInput").ap(),
        "WHH": nc.dram_tensor("WHH", [H, 2, G4], BF16, kind="ExternalInput").ap(),
        "ONES": nc.dram_tensor("ONES", [1, R], BF16, kind="ExternalInput").ap(),
        "DW": nc.dram_tensor("DW", [D, 2 * H], F32, kind="ExternalInput").ap(),
        "SEL": nc.dram_tensor("SEL", [J, BLOC], F32, kind="ExternalInput").ap(),
        "IDN": nc.dram_tensor("IDN", [H, H], BF16, kind="ExternalInput").ap(),
        "SELT": nc.dram_tensor("SELT", [BLOC, J], F32, kind="ExternalInput").ap(),
        "DB": nc.dram_tensor("DB", [D, 2 * H], F32, kind="ExternalInput").ap(),
    }
    outs = {
        "OUT": nc.dram_tensor("OUT", [J, 2 * H], F32, kind="ExternalOutput").ap(),
    }
    with tile.TileContext(nc) as tc:
        emit(tc, ins, outs)
    nc.compile()
    return nc


def _prep_dir(Wih, Whh, bih, bhh):
    # gate order (i,f,o,g); the g block is pre-scaled by 2 so the kernel can
    # evaluate tanh(g) as 2*sigmoid(2g)-1 inside the fused sigmoid op
    wihT = Wih.T.reshape(TS, 4, H)[:, PERM, :].reshape(TS, G4).copy()
    biasr = (bih + bhh).reshape(4, H)[PERM, :].reshape(G4).copy()
    wihT[:, 3 * H :] *= 2.0
    biasr[3 * H :] *= 2.0
    wih65 = np.concatenate([wihT, biasr[None, :]], axis=0).astype(BF16NP)
    whhT = Whh.T.reshape(H, 4, H)[:, PERM, :].reshape(H, G4).copy()
    whhT[:, 3 * H :] *= 2.0
    whhT = whhT.astype(BF16NP)
    return wih65, whhT


def prep_inputs(X, W_t, b_t, Wih_f, Whh_f, bih_f, bhh_f,
                Wih_b, Whh_b, bih_b, bhh_b, diag_w, diag_b):
    wih_f, whh_f = _prep_dir(Wih_f, Whh_f, bih_f, bhh_f)
    wih_b, whh_b = _prep_dir(Wih_b, Whh_b, bih_b, bhh_b)
    shared = {
        "WTT": np.ascontiguousarray(W_t.T, dtype=BF16NP),
        "BT": np.ascontiguousarray(b_t.reshape(TS, 1), dtype=np.float32),
        "WIH": np.ascontiguousarray(np.stack([wih_f, wih_b], axis=1)),
        "WHH": np.ascontiguousarray(np.stack([whh_f, whh_b], axis=1)),
        "ONES": np.ones((1, R), dtype=BF16NP),
        "SEL": np.kron(np.eye(BLOC, dtype=np.float32), np.ones((D, 1), np.float32)),
        "IDN": np.eye(H, dtype=BF16NP),
        "SELT": np.kron(np.eye(BLOC, dtype=np.float32), np.ones((1, D), np.float32)),
        "DW": np.ascontiguousarray(diag_w.reshape(D, 2 * H), dtype=np.float32),
        "DB": np.ascontiguousarray(diag_b.reshape(D, 2 * H), dtype=np.float32),
    }
    in_maps = []
    for i in range(NCORES):
        xt = np.ascontiguousarray(
            X[i * BLOC : (i + 1) * BLOC].transpose(3, 1, 0, 2).reshape(NF, R),
            dtype=BF16NP,
        )
        m = {"XT": xt}
        m.update(shared)
        in_maps.append(m)
    return in_maps


def kernel(**inputs):
    inputs = {k: np.asarray(v, dtype=np.float32) for k, v in inputs.items()}
    in_maps = prep_inputs(**inputs)
    nc = build_program()
    res = run_bass_kernel_spmd(nc, in_maps, list(range(NCORES)))
    out = np.concatenate(
        [res.results[i]["OUT"].reshape(BLOC, D, 2 * H) for i in range(NCORES)],
        axis=0,
    )
    return np.ascontiguousarray(out, dtype=np.float32)


if __name__ == "__main__":
    nc = build_program()
    print("program built ok")


# revision 18
# speedup vs baseline: 1.0134x; 1.0134x over previous
"""Trainium2 Bass kernel for nn_ContextEncoder.

Pipeline (per sample b): feature transform tanh(X @ W_t.T + b_t), a
"bidirectional" LSTM where both directions run forward (matching the
reference), attention pooling against the last hidden state, and a
context norm over the flattened (d, 2h) vector.

Sharding: data-parallel over b (16 samples -> 2 per core on 8 cores).
Each core runs 128 independent sequences (2 b x 64 d) of length T=128.

v2 schedule notes:
  - per-step PSUM gate groups (1 bank per dir, bufs=2); xW matmuls for
    step t+1 are emitted during step t so the PE never waits on the
    WAR hazard against the sigmoid reads.
  - PE transposes of h_t (for the attention tail) are emitted during
    step t+1, after the recurrence matmuls, keeping them off the
    critical h -> matmul path. They accumulate 4 steps per PSUM bank
    and are evacuated to the [j, t, 2h] slab by the GpSimd engine.
  - sigma_f * c_prev runs on GpSimd (off the serial path); the tg/u/cn
    chain stays on DVE.
  - tail: logits via DVE/GpSimd-split multiply + pair tree + packed
    tensor_reduce (contiguous fp32 logits), softmax on ACT, weighted
    pooling split across DVE/ACT/GpSimd, final reduction via
    tensor_reduce over a transposed view.
"""

import sys

for _p in ("/opt/trn_rl_repo", "/root/.axon_site/_ro/trn_rl_repo"):
    if _p not in sys.path:
        sys.path.append(_p)

import numpy as np

import concourse.bass as bass
import concourse.bacc as bacc
import concourse.tile as tile
from concourse import mybir
from concourse.bass_utils import run_bass_kernel_spmd

BF16NP = np.float16
F32 = mybir.dt.float32
BF16 = mybir.dt.float16
AF = mybir.ActivationFunctionType
ALU = mybir.AluOpType

B, T, D, NF = 16, 128, 64, 32
TS, H = 64, 128
NCORES = 8
BLOC = B // NCORES          # 2 samples per core
J = BLOC * D                # 128 sequences per core
R = J * T                   # 16384 (t, b, d) columns
G4 = 4 * H                  # 512 gates per direction
PERM = (0, 1, 3, 2)         # torch gate order (i,f,g,o) -> (i,f,o,g)
NORM_N = D * 2 * H          # 16384 context-norm elements per sample
TCH = 1024                  # transform chunk columns
# tail work split: DVE ~3.8x faster than GpSimd per element
TD = 104                    # t-range handled by DVE in prod/tree ops


def emit(tc, ins, outs):
    nc = tc.nc
    XT, WTT, BT = ins["XT"], ins["WTT"], ins["BT"]
    WIH, WHH, ONES = ins["WIH"], ins["WHH"], ins["ONES"]
    DW, DB = ins["DW"], ins["DB"]
    OUT = outs["OUT"]

    with (
        tc.tile_pool(name="consts", bufs=1) as consts,
        tc.tile_pool(name="sgpool", bufs=2) as sgpool,
        tc.tile_pool(name="small", bufs=2) as small,
    ):
        # ---- constants / weights ----
        wtt = consts.tile([NF, TS], BF16)
        nc.sync.dma_start(wtt, WTT)
        bt = consts.tile([TS, 1], F32)
        nc.sync.dma_start(bt, BT)
        wih = consts.tile([TS + 1, 2, G4], BF16)
        whh = consts.tile([H, 2, G4], BF16)
        # HT: attention layout [j, t, 2h] filled by per-step PE transposes
        ht = consts.tile([J, T, 2 * H], BF16)
        idn = consts.tile([H, H], BF16)

        with (
            tc.tile_pool(name="xs2p", bufs=1) as xs2p,
            tc.tile_pool(name="hslabs", bufs=1) as hslabs,
            tc.tile_pool(name="xtp", bufs=2) as xtp,
        ):
            # ---- feature transform: xs2[0:64, (t,b,d)] = tanh(Wt @ X.T + bt)
            # Computed in 512-col (4-step) chunks, interleaved into the
            # recurrence: only chunks 0-2 run up front.
            xs2 = xs2p.tile([TS + 1, R], BF16)
            BLK = 4096
            xts = {}

            def emit_xt_dma(blk):
                xt = xtp.tile([NF, BLK], BF16, tag="xt")
                nc.sync.dma_start(xt, XT[:, blk * BLK : (blk + 1) * BLK])
                xts[blk] = xt

            emit_xt_dma(0)
            nc.sync.dma_start(xs2[TS : TS + 1, :], ONES)
            nc.sync.dma_start(wih, WIH)
            nc.sync.dma_start(whh, WHH)
            nc.sync.dma_start(idn, ins["IDN"])
            # h history slabs: transpose sources are never recycled
            hsl = [hslabs.tile([H, R], BF16, tag=f"hs{d}", name=f"hs{d}")
                   for d in range(2)]
            # per-dir [parity, {tg, c}] state tiles: uc2 reads (tg@p, c@1-p)
            # as one paged AP, pairing with sigma chunks (i, f)
            ctile = [hslabs.tile([H, 2, 2, J], BF16, tag=f"ct{d}",
                                 name=f"ct{d}") for d in range(2)]
            h_prev = [None, None]
            for d in range(2):
                h0 = hslabs.tile([H, J], BF16, tag=f"hz{d}", name=f"hz{d}")
                nc.vector.memset(h0, 0.0)
                nc.vector.memset(ctile[d][:, 1, 1, :], 0.0)
                h_prev[d] = h0

            # ---- recurrence ----
            # Per-step PSUM gate tiles (1 bank/dir, double buffered).
            # trp: 4 steps x 2 dirs of transposed h per bank.
            with (
                tc.tile_pool(name="gates", bufs=2, space="PSUM") as gates,
                tc.tile_pool(name="trp", bufs=3, space="PSUM") as trp,
                tc.tile_pool(name="tfp", bufs=1, space="PSUM") as tfp,
            ):
                def emit_tf_mm(c):
                    pz = tfp.tile([TS, 512], F32, tag="pz")
                    nc.tensor.matmul(
                        pz, lhsT=wtt,
                        rhs=xts[c // 8][:, (c % 8) * 512 : (c % 8 + 1) * 512],
                        start=True, stop=True)
                    return pz

                def emit_tf_act(c, pz):
                    nc.scalar.activation(
                        out=xs2[0:TS, c * 512 : (c + 1) * 512],
                        in_=pz, func=AF.Tanh, bias=bt, scale=1.0)

                def emit_xw(t, pg):
                    rhs_x = xs2[:, t * J : (t + 1) * J]
                    for d in range(2):
                        for c in range(4):
                            nc.tensor.matmul(
                                pg[d][:, c, :],
                                lhsT=wih[:, d, c * H : (c + 1) * H],
                                rhs=rhs_x, start=(c == 0), stop=False,
                            )

                def new_group():
                    return [gates.tile([H, 4, J], F32, tag=f"g{d}",
                                       name=f"pg{d}") for d in range(2)]

                for c in range(3):
                    emit_tf_act(c, emit_tf_mm(c))
                psg = new_group()
                emit_xw(0, psg)
                psg_next = None
                trtile = [None]
                tr_keep = []
                for t in range(T):
                    # prefetch the next XT block / transform chunk state
                    if t in (8, 40, 72):
                        emit_xt_dma(t // 32 + 1)
                    tf_c = (t + 7) // 4 if (t % 4 == 1) else None
                    if tf_c is not None and not (3 <= tf_c < 32):
                        tf_c = None
                    # recurrence matmuls for step t
                    for d in range(2):
                        for c in range(4):
                            nc.tensor.matmul(
                                psg[d][:, c, :],
                                lhsT=whh[:, d, c * H : (c + 1) * H],
                                rhs=h_prev[d], start=False, stop=(c == 3),
                            )
                    # xW for step t+1 fills the PE while ACT/DVE work on t
                    if t < T - 1:
                        psg_next = new_group()
                        emit_xw(t + 1, psg_next)
                    # transposes of h_{t-1} (off critical path, last on PE)
                    if t >= 1:
                        s = t - 1
                        if s % 4 == 0:
                            trtile[0] = trp.tile([J, 8, H], BF16, tag="tr",
                                                 name="ptr")
                            tr_keep.append(trtile[0])
                        for d in range(2):
                            nc.tensor.transpose(
                                trtile[0][:, (s % 4) * 2 + d, :],
                                hsl[d][:, s * J : (s + 1) * J], idn)
                    tf_pz = emit_tf_mm(tf_c) if tf_c is not None else None

                    # elementwise tail of step t
                    p = t % 2
                    sg4, uc2, tc_ = {}, {}, {}
                    for d in range(2):
                        sg4[d] = sgpool.tile([H, 4, J], BF16, tag=f"sg{d}",
                                             name=f"sg{d}")
                        nc.scalar.activation(out=sg4[d], in_=psg[d],
                                             func=AF.Sigmoid)
                    for d in range(2):
                        # tg = 2*sigmoid(2g) - 1 into the parity-p tg slot
                        nc.vector.tensor_scalar(ctile[d][:, p, 0, :],
                                                sg4[d][:, 3, :], 2.0, -1.0,
                                                op0=ALU.mult, op1=ALU.add)
                    for d in range(2):
                        # c2 = sg_f * c_prev (off the serial path)
                        uc2[d] = small.tile([H, 2, J], BF16, tag=f"uc{d}",
                                            name=f"uc{d}")
                        nc.vector.tensor_mul(uc2[d][:, 1, :], sg4[d][:, 1, :],
                                             ctile[d][:, 1 - p, 1, :])
                    for d in range(2):
                        # u = sg_i * tg (serial path)
                        nc.vector.tensor_mul(uc2[d][:, 0, :], sg4[d][:, 0, :],
                                             ctile[d][:, p, 0, :])
                    for d in range(2):
                        nc.vector.tensor_add(ctile[d][:, p, 1, :],
                                             uc2[d][:, 0, :], uc2[d][:, 1, :])
                    for d in range(2):
                        tc_[d] = small.tile([H, J], BF16, tag=f"tc{d}",
                                            name=f"tc{d}")
                        nc.scalar.activation(out=tc_[d], in_=ctile[d][:, p, 1, :],
                                             func=AF.Tanh)
                    # evacuate two steps of transposed h on the (in-order)
                    # scalar engine right after tanh: lands in ACT's idle
                    # window each even step, never preempting the DVE chain
                    # evacuate two steps of transposed h on the (in-order)
                    # scalar engine right after tanh: lands in ACT's idle
                    # window each even step, never preempting the DVE chain
                    if t >= 2 and t % 2 == 0:
                        sl = ((t - 2) % 4) * 2
                        nc.scalar.copy(
                            ht[:, t - 2 : t, :].rearrange("j a b -> j (a b)"),
                            trtile[0][:, sl : sl + 4, :].rearrange(
                                "j a b -> j (a b)"),
                        )
                    if tf_c is not None:
                        emit_tf_act(tf_c, tf_pz)
                    for d in range(2):
                        hn = hsl[d][:, t * J : (t + 1) * J]
                        nc.vector.tensor_mul(hn, sg4[d][:, 2, :], tc_[d])
                        h_prev[d] = hn
                    psg = psg_next
                # final transposes for step T-1 (slot s%4 = 3 of current tile)
                s = T - 1
                for d in range(2):
                    nc.tensor.transpose(trtile[0][:, (s % 4) * 2 + d, :],
                                        hsl[d][:, s * J : (s + 1) * J], idn)
                nc.scalar.copy(
                    ht[:, T - 2 : T, :].rearrange("j a b -> j (a b)"),
                    trtile[0][:, 4:8, :].rearrange("j a b -> j (a b)"),
                )

        # ---- tail: attention pooling + context norm ----
        with (
            tc.tile_pool(name="tailp", bufs=1) as tailp,
            tc.tile_pool(name="tailps", bufs=1, space="PSUM") as tailps,
        ):
            htj = ht[:, T - 1, :]  # [J, 2H] last hidden state
            def htj_bcast(t0, t1):
                return bass.AP(
                    tensor=htj.tensor, offset=htj.offset,
                    ap=[list(htj.ap[0]), [0, t1 - t0], list(htj.ap[-1])],
                )
            prod = tailp.tile([J, T, 2 * H], BF16)
            nc.vector.tensor_mul(prod, ht, htj_bcast(0, T))
            # pair tree over the 2h dim, bf16, t-split DVE/GpSimd
            pp0 = tailp.tile([J, T, 128], BF16)
            def lvl(dst, a, b):
                nc.vector.tensor_add(dst, a, b)
            lvl(pp0, prod[:, :, 0:128], prod[:, :, 128:256])
            lvl(prod[:, :, 0:64], pp0[:, :, 0:64], pp0[:, :, 64:128])
            lvl(pp0[:, :, 0:32], prod[:, :, 0:32], prod[:, :, 32:64])
            lvl(prod[:, :, 0:16], pp0[:, :, 0:16], pp0[:, :, 16:32])
            # packed fp32 logits via innermost tensor_reduce
            lt = tailp.tile([J, T], F32)
            nc.vector.tensor_reduce(lt, prod[:, :, 0:16],
                                    axis=mybir.AxisListType.X, op=ALU.add)
            mx = tailp.tile([J, 1], F32)
            nc.vector.tensor_reduce(mx, lt, axis=mybir.AxisListType.X, op=ALU.max)
            mxn = tailp.tile([J, 1], F32)
            nc.vector.tensor_scalar_mul(mxn, mx, -1.0)
            ew = tailp.tile([J, T], F32)
            dsum = tailp.tile([J, 1], F32)
            nc.scalar.activation(out=ew, in_=lt, func=AF.Exp, bias=mxn,
                                 scale=1.0, accum_out=dsum)
            rd = tailp.tile([J, 1], F32)
            nc.vector.reciprocal(rd, dsum)
            nc.vector.tensor_scalar_mul(ew, ew, rd)  # softmax weights in place
            # weighted pooling: per-t scale ops split DVE/ACT/GpSimd
            prod2 = tailp.tile([J, T, 2 * H], BF16, tag="prod")  # reuse slab
            act_ts = set(range(43, 64)) | set(range(107, 128))
            for tt in list(range(0, 43)) + list(range(64, 107)) + sorted(act_ts):
                if tt in act_ts:
                    nc.scalar.mul(prod2[:, tt, :], ht[:, tt, :],
                                  ew[:, tt : tt + 1])
                else:
                    nc.vector.tensor_scalar_mul(prod2[:, tt, :], ht[:, tt, :],
                                                ew[:, tt : tt + 1])
            # pair tree over t: bf16 down to 16 steps, then fp32
            def tlvl(dst, a, b, n):
                nc.vector.tensor_add(dst, a, b)
            s64 = pp0.rearrange("j t b -> j (t b)").rearrange(
                "j (t b) -> j t b", b=2 * H)  # [J, 64, 2H] view over pp0
            nc.vector.tensor_add(s64[:, 0:43, :], prod2[:, 0:43, :],
                                 prod2[:, 64:107, :])
            nc.vector.tensor_add(s64[:, 43:64, :], prod2[:, 43:64, :],
                                 prod2[:, 107:128, :])
            tlvl(prod2[:, 0:32, :], s64[:, 0:32, :], s64[:, 32:64, :], 32)
            tlvl(s64[:, 0:16, :], prod2[:, 0:16, :], prod2[:, 16:32, :], 16)
            ptrf = tailp.tile([J, 8, 2 * H], F32)
            tlvl(ptrf, s64[:, 0:8, :], s64[:, 8:16, :], 8)
            nc.vector.tensor_add(ptrf[:, 0:4, :], ptrf[:, 0:4, :],
                                 ptrf[:, 4:8, :])
            nc.vector.tensor_add(ptrf[:, 0:2, :], ptrf[:, 0:2, :],
                                 ptrf[:, 2:4, :])
            pooled = tailp.tile([J, 2 * H], F32)
            nc.vector.tensor_add(pooled, ptrf[:, 0, :], ptrf[:, 1, :])

            # context norm across each sample's (d, 2h) block
            pooled2 = tailp.tile([J, 2 * H], F32)
            nc.scalar.activation(out=pooled2, in_=pooled, func=AF.Square)
            sel = tailp.tile([J, BLOC], F32)
            nc.sync.dma_start(sel, ins["SEL"])
            pstat = tailps.tile([BLOC, 2 * G4], F32, tag="stats")
            nc.tensor.matmul(pstat[:, 0 : 2 * H], lhsT=sel, rhs=pooled,
                             start=True, stop=False)
            nc.tensor.matmul(pstat[:, 2 * H : 4 * H], lhsT=sel, rhs=pooled2,
                             start=False, stop=True)
            s1 = tailp.tile([BLOC, 1], F32)
            nc.vector.tensor_reduce(s1, pstat[:, 0 : 2 * H],
                                    axis=mybir.AxisListType.X, op=ALU.add)
            s2 = tailp.tile([BLOC, 1], F32)
            nc.vector.tensor_reduce(s2, pstat[:, 2 * H : 4 * H],
                                    axis=mybir.AxisListType.X, op=ALU.add)
            stats2 = tailp.tile([BLOC, 2], F32)
            nc.scalar.mul(stats2[:, 0:1], s1, 1.0 / NORM_N)      # mean
            q = tailp.tile([BLOC, 1], F32)
            nc.vector.tensor_mul(q, s1, stats2[:, 0:1])          # sum*mean
            v = tailp.tile([BLOC, 1], F32)
            nc.vector.tensor_tensor(v, s2, q, op=ALU.subtract)
            sd = tailp.tile([BLOC, 1], F32)
            nc.scalar.activation(out=sd, in_=v, func=AF.Sqrt,
                                 scale=1.0 / (NORM_N - 1))
            nc.vector.reciprocal(stats2[:, 1:2], sd)
            selt = tailp.tile([BLOC, J], F32)
            nc.sync.dma_start(selt, ins["SELT"])
            pmb = tailps.tile([J, 2], F32, tag="mb")
            nc.tensor.matmul(pmb, lhsT=selt, rhs=stats2, start=True, stop=True)
            mb = tailp.tile([J, 2], F32)
            nc.vector.tensor_copy(mb, pmb)
            dwt = tailp.tile([J, 2 * H], F32)
            nc.sync.dma_start(dwt[0:D, :], DW)
            nc.sync.dma_start(dwt[D:J, :], DW)
            dbt = tailp.tile([J, 2 * H], F32)
            nc.sync.dma_start(dbt[0:D, :], DB)
            nc.sync.dma_start(dbt[D:J, :], DB)
            t1 = tailp.tile([J, 2 * H], F32)
            nc.vector.tensor_scalar(t1, pooled, mb[:, 0:1], mb[:, 1:2],
                                    op0=ALU.subtract, op1=ALU.mult)
            t2 = tailp.tile([J, 2 * H], F32)
            nc.vector.tensor_mul(t2, t1, dwt)
            t3 = tailp.tile([J, 2 * H], F32)
            nc.vector.tensor_add(t3, t2, dbt)
            nc.sync.dma_start(OUT, t3)


def build_program():
    nc = bacc.Bacc("TRN2", target_bir_lowering=False, debug=False)
    ins = {
        "XT": nc.dram_tensor("XT", [NF, R], BF16, kind="ExternalInput").ap(),
        "WTT": nc.dram_tensor("WTT", [NF, TS], BF16, kind="ExternalInput").ap(),
        "BT": nc.dram_tensor("BT", [TS, 1], F32, kind="ExternalInput").ap(),
        "WIH": nc.dram_tensor("WIH", [TS + 1, 2, G4], BF16, kind="ExternalInput").ap(),
        "WHH": nc.dram_tensor("WHH", [H, 2, G4], BF16, kind="ExternalInput").ap(),
        "ONES": nc.dram_tensor("ONES", [1, R], BF16, kind="ExternalInput").ap(),
        "DW": nc.dram_tensor("DW", [D, 2 * H], F32, kind="ExternalInput").ap(),
        "SEL": nc.dram_tensor("SEL", [J, BLOC], F32, kind="ExternalInput").ap(),
        "IDN": nc.dram_tensor("IDN", [H, H], BF16, kind="ExternalInput").ap(),
        "SELT": nc.dram_tensor("SELT", [BLOC, J], F32, kind="ExternalInput").ap(),
        "DB": nc.dram_tensor("DB", [D, 2 * H], F32, kind="ExternalInput").ap(),
    }
    outs = {
        "OUT": nc.dram_tensor("OUT", [J, 2 * H], F32, kind="ExternalOutput").ap(),
    }
    with tile.TileContext(nc) as tc:
        emit(tc, ins, outs)
    nc.compile()
    return nc


def _prep_dir(Wih, Whh, bih, bhh):
    # gate order (i,f,o,g); the g block is pre-scaled by 2 so the kernel can
    # evaluate tanh(g) as 2*sigmoid(2g)-1 inside the fused sigmoid op
    wihT = Wih.T.reshape(TS, 4, H)[:, PERM, :].reshape(TS, G4).copy()
    biasr = (bih + bhh).reshape(4, H)[PERM, :].reshape(G4).copy()
    wihT[:, 3 * H :] *= 2.0
    biasr[3 * H :] *= 2.0
    wih65 = np.concatenate([wihT, biasr[None, :]], axis=0).astype(BF16NP)
    whhT = Whh.T.reshape(H, 4, H)[:, PERM, :].reshape(H, G4).copy()
    whhT[:, 3 * H :] *= 2.0
    whhT = whhT.astype(BF16NP)
    return wih65, whhT


def prep_inputs(X, W_t, b_t, Wih_f, Whh_f, bih_f, bhh_f,
                Wih_b, Whh_b, bih_b, bhh_b, diag_w, diag_b):
    wih_f, whh_f = _prep_dir(Wih_f, Whh_f, bih_f, bhh_f)
    wih_b, whh_b = _prep_dir(Wih_b, Whh_b, bih_b, bhh_b)
    shared = {
        "WTT": np.ascontiguousarray(W_t.T, dtype=BF16NP),
        "BT": np.ascontiguousarray(b_t.reshape(TS, 1), dtype=np.float32),
        "WIH": np.ascontiguousarray(np.stack([wih_f, wih_b], axis=1)),
        "WHH": np.ascontiguousarray(np.stack([whh_f, whh_b], axis=1)),
        "ONES": np.ones((1, R), dtype=BF16NP),
        "SEL": np.kron(np.eye(BLOC, dtype=np.float32), np.ones((D, 1), np.float32)),
        "IDN": np.eye(H, dtype=BF16NP),
        "SELT": np.kron(np.eye(BLOC, dtype=np.float32), np.ones((1, D), np.float32)),
        "DW": np.ascontiguousarray(diag_w.reshape(D, 2 * H), dtype=np.float32),
        "DB": np.ascontiguousarray(diag_b.reshape(D, 2 * H), dtype=np.float32),
    }
    in_maps = []
    for i in range(NCORES):
        xt = np.ascontiguousarray(
            X[i * BLOC : (i + 1) * BLOC].transpose(3, 1, 0, 2).reshape(NF, R),
            dtype=BF16NP,
        )
        m = {"XT": xt}
        m.update(shared)
        in_maps.append(m)
    return in_maps


def kernel(**inputs):
    inputs = {k: np.asarray(v, dtype=np.float32) for k, v in inputs.items()}
    in_maps = prep_inputs(**inputs)
    nc = build_program()
    res = run_bass_kernel_spmd(nc, in_maps, list(range(NCORES)))
    out = np.concatenate(
        [res.results[i]["OUT"].reshape(BLOC, D, 2 * H) for i in range(NCORES)],
        axis=0,
    )
    return np.ascontiguousarray(out, dtype=np.float32)


if __name__ == "__main__":
    nc = build_program()
    print("program built ok")
